# revision 15
# baseline (speedup 1.0000x reference)
"""Trainium2 Bass kernel for additive (Bahdanau-style) attention with length masking.

Reference computation (per batch b):
    q[b]      = W1 @ last_hidden[b] + attn_b                  # [H]
    k[l,b]    = W2 @ encoder_outputs[l,b]                     # [H]
    score[b,l]= v . tanh(q[b] + k[l,b])                       # masked to l < src_len[b]
    attn[b]   = softmax_l(score[b])                           # [L]
    ctx[b]    = sum_l attn[b,l] * encoder_outputs[l,b]        # [H]
returns (ctx [B,1,H], attn [B,1,L])

Sharding: data-parallel over batch. 128 batches -> 8 cores x 16 slots.
Batches are sorted by ceil(src_len/128) and dealt rank-major so all cores
run one identical program with per-slot tile counts C[s]; rows beyond
src_len are masked, tiles beyond ceil(src_len/128) are skipped entirely.

Precision: the score matmul runs as a two-term compensated bf16 product
    e @ W2^T ~= eh@Wh + eh@Wl + el@Wh      (eh,el / Wh,Wl bf16 hi/lo splits)
because this hardware path supports only bf16 (1 cyc/row) and fp32
(4 cyc/row) matmuls -- fp16 / f32r / int16 all fail in walrus or on
silicon. Everything after the matmul stays fp32. The q projection uses
the same compensated split (one-time). The q+bias row rides inside the
third K-chunk of the score matmul via a ones-row appended to the
transposed operand (host-prepared).
"""

import sys
import os

if "/opt/trn_rl_repo" not in sys.path:
    sys.path.insert(0, "/opt/trn_rl_repo")

import numpy as np
import ml_dtypes

from concourse import bass, bacc, tile, mybir
from concourse.bass_utils import run_bass_kernel_spmd

F32 = mybir.dt.float32
BF16 = mybir.dt.bfloat16
AF = mybir.ActivationFunctionType
ALU = mybir.AluOpType

H = 300
L = 2048
B = 128
NCORES = 8
BL = B // NCORES        # batches (slots) per core
TP = 128                # rows per tile
NEG = -1.0e9


def _build_program(C, mask_col, n_mask, nb_enc):
    nc = bacc.Bacc("TRN2", target_bir_lowering=False, debug=False)

    enc_d = nc.declare_dram_parameter("enc16", [BL, L, H], BF16, isOutput=False)
    ehT_d = nc.declare_dram_parameter("ehT", [BL, H + 1, L], BF16, isOutput=False)
    elT_d = nc.declare_dram_parameter("elT", [BL, H, L], BF16, isOutput=False)
    w2h01_d = nc.declare_dram_parameter("w2h01", [256, H], BF16, isOutput=False)
    w2h2_d = nc.declare_dram_parameter("w2h2", [44, H], BF16, isOutput=False)
    w2l01_d = nc.declare_dram_parameter("w2l01", [256, H], BF16, isOutput=False)
    w2l2_d = nc.declare_dram_parameter("w2l2", [44, H], BF16, isOutput=False)
    w1h_d = nc.declare_dram_parameter("w1h", [H, H], BF16, isOutput=False)
    w1l_d = nc.declare_dram_parameter("w1l", [H, H], BF16, isOutput=False)
    lhh_d = nc.declare_dram_parameter("lhh", [H, BL], BF16, isOutput=False)
    lhl_d = nc.declare_dram_parameter("lhl", [H, BL], BF16, isOutput=False)
    bias_d = nc.declare_dram_parameter("bias_row", [1, H], F32, isOutput=False)
    vbc_d = nc.declare_dram_parameter("vbc", [128, H], F32, isOutput=False)
    ident_d = nc.declare_dram_parameter("ident", [128, 128], F32, isOutput=False)
    ones_d = nc.declare_dram_parameter("ones128", [128, 128], F32, isOutput=False)
    pen_d = nc.declare_dram_parameter("pen", [128, n_mask], F32, isOutput=False)

    ctx_d = nc.declare_dram_parameter("ctx_out", [BL, H], F32, isOutput=True)
    attn_d = nc.declare_dram_parameter("attn_out", [BL, L], F32, isOutput=True)

    with tile.TileContext(nc) as tc:
        with (
            tc.tile_pool(name="const", bufs=1) as constp,
            tc.tile_pool(name="s3", bufs=1) as s3p,
            tc.tile_pool(name="encring", bufs=1) as encp,
            tc.tile_pool(name="encT", bufs=3) as encTp,
            tc.tile_pool(name="th", bufs=3) as thp,
            tc.tile_pool(name="ttr", bufs=2) as ttrp,
            tc.tile_pool(name="scores", bufs=2) as scorep,
            tc.tile_pool(name="wts", bufs=2) as wp,
            tc.tile_pool(name="small", bufs=4) as smallp,
            tc.tile_pool(name="mm1ps", bufs=2, space="PSUM") as mm1ps,
            tc.tile_pool(name="ctxps", bufs=2, space="PSUM") as ctxps,
            tc.tile_pool(name="sps", bufs=3, space="PSUM") as sps,
        ):
            # ---- constants ----
            w2h0 = constp.tile([128, H], BF16, tag="w2h0")
            w2h1 = constp.tile([128, H], BF16, tag="w2h1")
            w2h2 = constp.tile([44, H], BF16, tag="w2h2")
            w2l0 = constp.tile([128, H], BF16, tag="w2l0")
            w2l1 = constp.tile([128, H], BF16, tag="w2l1")
            vbc = constp.tile([128, H], F32, tag="vbc")
            ident = constp.tile([128, 128], F32, tag="ident")
            ones = constp.tile([128, 128], F32, tag="ones")
            pen = constp.tile([128, n_mask], F32, tag="pen")
            w1h = [constp.tile([min(128, H - 128 * i), H], BF16,
                               tag=f"w1h{i}", name=f"w1h{i}") for i in range(3)]
            w1l = [constp.tile([min(128, H - 128 * i), H], BF16,
                               tag=f"w1l{i}", name=f"w1l{i}") for i in range(3)]
            lhh = [constp.tile([min(128, H - 128 * i), BL], BF16,
                               tag=f"lhh{i}", name=f"lhh{i}") for i in range(3)]
            lhl = [constp.tile([min(128, H - 128 * i), BL], BF16,
                               tag=f"lhl{i}", name=f"lhl{i}") for i in range(3)]
            biasr = constp.tile([1, H], F32, tag="biasr")
            qh16 = constp.tile([BL, H], BF16, tag="qh16")
            ql16 = constp.tile([BL, H], BF16, tag="ql16")

            nc.sync.dma_start(w2h0[:], w2h01_d[0:128, :])
            nc.sync.dma_start(w2h1[:], w2h01_d[128:256, :])
            nc.sync.dma_start(w2l0[:], w2l01_d[0:128, :])
            nc.sync.dma_start(w2l1[:], w2l01_d[128:256, :])
            for i, (k0, k1) in enumerate([(0, 128), (128, 256), (256, 300)]):
                nc.sync.dma_start(w1h[i][:], w1h_d[k0:k1, :])
                nc.sync.dma_start(w1l[i][:], w1l_d[k0:k1, :])
                nc.sync.dma_start(lhh[i][:], lhh_d[k0:k1, :])
                nc.sync.dma_start(lhl[i][:], lhl_d[k0:k1, :])
            for tl, dd in [(w2h2, w2h2_d), (vbc, vbc_d), (ident, ident_d),
                           (ones, ones_d), (pen, pen_d), (biasr, bias_d)]:
                nc.sync.dma_start(tl[:], dd[:])

            # slot-alternating [45, H] rhs blocks for K-chunk 3:
            # s3h rows 0:44 = W2h tail, row 44 = qh[slot]
            # s3l rows 0:44 = W2l tail, row 44 = ql[slot]
            s3h = [s3p.tile([45, H], BF16, tag=f"s3h_{i}", name=f"s3h_{i}")
                   for i in range(2)]
            s3l = [s3p.tile([45, H], BF16, tag=f"s3l_{i}", name=f"s3l_{i}")
                   for i in range(2)]
            for i in range(2):
                nc.sync.dma_start(s3h[i][0:44, :], w2h2_d[:])
                nc.sync.dma_start(s3l[i][0:44, :], w2l2_d[:])

            # ---- q = last_hidden @ W1^T + attn_b, compensated bf16 ----
            q_ps = sps.tile([BL, H], F32, tag="sp")
            first = True
            for (lh_t, w_t) in [(lhh, w1h), (lhh, w1l), (lhl, w1h)]:
                for i in range(3):
                    nc.tensor.matmul(q_ps[:], lh_t[i][:], w_t[i][:],
                                     start=first, stop=False)
                    first = False
            nc.tensor.matmul(q_ps[:], ones[0:1, 0:BL], biasr[:],
                             start=False, stop=True)
            q32 = constp.tile([BL, H], F32, tag="q32")
            nc.vector.tensor_copy(q32[:], q_ps[:])
            nc.vector.tensor_copy(qh16[:], q32[:])
            qh32 = constp.tile([BL, H], F32, tag="qh32")
            nc.vector.tensor_copy(qh32[:], qh16[:])
            ql32 = constp.tile([BL, H], F32, tag="ql32")
            nc.vector.tensor_tensor(out=ql32[:], in0=q32[:], in1=qh32[:],
                                    op=ALU.subtract)
            nc.vector.tensor_copy(ql16[:], ql32[:])

            # ---- native-layout enc ring (resident for the ctx matmul) ----
            ebs16 = [encp.tile([TP, H], BF16, tag=f"eb16_{i}", name=f"eb16_{i}")
                     for i in range(nb_enc)]

            gidx = 0
            tile_of = {}
            for s in range(BL):
                cs = C[s]
                s3hb = s3h[s % 2]
                s3lb = s3l[s % 2]
                nc.sync.dma_start(s3hb[44:45, :], qh16[s:s + 1, :])
                nc.sync.dma_start(s3lb[44:45, :], ql16[s:s + 1, :])

                scores = scorep.tile([128, BL], F32, tag="scores")
                for t in range(cs):
                    eb16 = ebs16[gidx % nb_enc]
                    tile_of[(s, t)] = eb16
                    gidx += 1
                    nc.sync.dma_start(eb16[:], enc_d[s][t * TP:(t + 1) * TP, :])
                    lcol = slice(t * TP, (t + 1) * TP)
                    eh0 = encTp.tile([128, 128], BF16, tag="eh0", name="eh0")
                    eh1 = encTp.tile([128, 128], BF16, tag="eh1", name="eh1")
                    eh2 = encTp.tile([45, 128], BF16, tag="eh2", name="eh2")
                    el0 = encTp.tile([128, 128], BF16, tag="el0", name="el0")
                    el1 = encTp.tile([128, 128], BF16, tag="el1", name="el1")
                    el2 = encTp.tile([44, 128], BF16, tag="el2", name="el2")
                    nc.sync.dma_start(eh0[:], ehT_d[s][0:128, lcol])
                    nc.sync.dma_start(eh1[:], ehT_d[s][128:256, lcol])
                    nc.sync.dma_start(eh2[:], ehT_d[s][256:301, lcol])
                    nc.sync.dma_start(el0[:], elT_d[s][0:128, lcol])
                    nc.sync.dma_start(el1[:], elT_d[s][128:256, lcol])
                    nc.sync.dma_start(el2[:], elT_d[s][256:300, lcol])
                    pe = mm1ps.tile([TP, H], F32, tag="mm1")
                    nc.tensor.matmul(pe[:], eh0[:], w2h0[:],
                                     start=True, stop=False)
                    nc.tensor.matmul(pe[:], eh1[:], w2h1[:],
                                     start=False, stop=False)
                    nc.tensor.matmul(pe[:], eh2[:], s3hb[0:45, :],
                                     start=False, stop=False)
                    nc.tensor.matmul(pe[:], eh0[:], w2l0[:],
                                     start=False, stop=False)
                    nc.tensor.matmul(pe[:], eh1[:], w2l1[:],
                                     start=False, stop=False)
                    nc.tensor.matmul(pe[:], eh2[:], s3lb[0:45, :],
                                     start=False, stop=False)
                    nc.tensor.matmul(pe[:], el0[:], w2h0[:],
                                     start=False, stop=False)
                    nc.tensor.matmul(pe[:], el1[:], w2h1[:],
                                     start=False, stop=False)
                    nc.tensor.matmul(pe[:], el2[:], w2h2[:],
                                     start=False, stop=True)
                    th = thp.tile([TP, H], F32, tag="th")
                    nc.scalar.activation(th[:], pe[:], AF.Tanh)
                    td = ttrp.tile([TP, H], F32, tag="ttr")
                    nc.vector.tensor_tensor(out=td[:], in0=th[:], in1=vbc[:],
                                            op=ALU.mult)
                    nc.vector.tensor_reduce(scores[:, t:t + 1], td[:],
                                            axis=mybir.AxisListType.X,
                                            op=ALU.add)
                    pcol = mask_col.get((s, t))
                    if pcol is not None:
                        nc.vector.tensor_tensor(
                            out=scores[:, t:t + 1], in0=scores[:, t:t + 1],
                            in1=pen[:, pcol:pcol + 1], op=ALU.add)

                # ---- softmax over this slot's cs*128 scores ----
                m1 = smallp.tile([128, 1], F32, tag="m1")
                nc.vector.tensor_reduce(m1[:], scores[:, 0:cs],
                                        axis=mybir.AxisListType.X, op=ALU.max)
                mT = sps.tile([1, 128], F32, tag="sp")
                nc.tensor.matmul(mT[:], m1[:], ident[:], is_transpose=True,
                                 start=True, stop=True)
                mmax = smallp.tile([1, 1], F32, tag="mmax")
                nc.vector.tensor_reduce(mmax[:], mT[:],
                                        axis=mybir.AxisListType.X, op=ALU.max)
                negM = smallp.tile([1, 1], F32, tag="negM")
                nc.vector.tensor_scalar_mul(negM[:], mmax[:], -1.0)
                nMb_ps = sps.tile([128, 1], F32, tag="sp")
                nc.tensor.matmul(nMb_ps[:], ones[0:1, :], negM[:],
                                 start=True, stop=True)
                negMb = smallp.tile([128, 1], F32, tag="negMb")
                nc.vector.tensor_copy(negMb[:], nMb_ps[:])

                wts = wp.tile([128, BL], F32, tag="wts")
                sume = smallp.tile([128, 1], F32, tag="sume")
                nc.scalar.activation(wts[:, 0:cs], scores[:, 0:cs], AF.Exp,
                                     bias=negMb[:], scale=1.0)
                nc.vector.tensor_reduce(sume[:], wts[:, 0:cs],
                                        axis=mybir.AxisListType.X, op=ALU.add)
                z_ps = sps.tile([1, 1], F32, tag="sp")
                nc.tensor.matmul(z_ps[:], ones[:, 0:1], sume[:],
                                 start=True, stop=True)
                zsb = smallp.tile([1, 1], F32, tag="zsb")
                nc.vector.tensor_copy(zsb[:], z_ps[:])
                rinv = smallp.tile([1, 1], F32, tag="rinv")
                nc.vector.reciprocal(rinv[:], zsb[:])
                rb_ps = sps.tile([128, 1], F32, tag="sp")
                nc.tensor.matmul(rb_ps[:], ones[0:1, :], rinv[:],
                                 start=True, stop=True)
                rinvb = smallp.tile([128, 1], F32, tag="rinvb")
                nc.vector.tensor_copy(rinvb[:], rb_ps[:])

                # ---- ctx[s] = (1/Z) * sum_t enc_tile^T @ w_tile (bf16 PE) ----
                w16 = wp.tile([128, BL], BF16, tag="w16")
                nc.vector.tensor_copy(w16[:, 0:cs], wts[:, 0:cs])
                cps = ctxps.tile([1, H], F32, tag="ctx")
                for t in range(cs):
                    eb16 = tile_of[(s, t)]
                    nc.tensor.matmul(
                        cps[:],
                        w16[:, t:t + 1],
                        eb16[:],
                        start=(t == 0), stop=(t == cs - 1),
                    )
                ctx_sb = smallp.tile([1, H], F32, tag="ctxsb")
                nc.scalar.activation(ctx_sb[:], cps[:], AF.Copy, scale=rinv[:])
                nc.sync.dma_start(ctx_d[s:s + 1, :], ctx_sb[:])

                # ---- attn weights out: normalize, transpose, store ----
                wn = wp.tile([128, BL], F32, tag="wn")
                nc.scalar.activation(wn[:, 0:cs], wts[:, 0:cs], AF.Copy,
                                     scale=rinvb[:])
                aT_ps = sps.tile([BL, 128], F32, tag="sp")
                nc.tensor.matmul(aT_ps[0:cs, :], wn[:, 0:cs], ident[:],
                                 is_transpose=True, start=True, stop=True)
                aT = smallp.tile([BL, 128], F32, tag="aT")
                nc.vector.tensor_copy(aT[0:cs, :], aT_ps[0:cs, :])
                nc.sync.dma_start(
                    attn_d[s].rearrange("(t l) -> t l", l=TP)[0:cs, :],
                    aT[0:cs, :],
                )

    nc.compile()
    return nc


def _split_bf16(x):
    bf = ml_dtypes.bfloat16
    hi = x.astype(bf)
    lo = (x - hi.astype(np.float32)).astype(bf)
    return hi, lo


def _prep(last_hidden, encoder_outputs, attn_W, attn_b, v, src_len):
    """Host-side sharding / marshaling."""
    bf = ml_dtypes.bfloat16
    src_len = np.asarray(src_len)
    tiles = -(-src_len // TP)                      # ceil
    order = np.argsort(-tiles, kind="stable")
    assign = np.zeros((NCORES, BL), dtype=np.int64)
    C = []
    for s in range(BL):
        grp = order[NCORES * s:NCORES * (s + 1)]
        assign[:, s] = grp
        C.append(int(tiles[grp].max()))

    mask_col = {}
    pen_cols = []
    for s in range(BL):
        for t in range(C[s]):
            needs = False
            for c in range(NCORES):
                b = assign[c, s]
                if (t + 1) * TP > src_len[b]:
                    needs = True
                    break
            if needs:
                mask_col[(s, t)] = len(pen_cols)
                pen_cols.append((s, t))
    n_mask = max(1, len(pen_cols))

    W1 = np.asarray(attn_W[:, :H], dtype=np.float32)   # [h_out, h_in]
    W2 = np.asarray(attn_W[:, H:], dtype=np.float32)
    W2T = np.ascontiguousarray(W2.T)                   # [h_in, h_out]
    W1T = np.ascontiguousarray(W1.T)
    W2Th, W2Tl = _split_bf16(W2T)
    W1Th, W1Tl = _split_bf16(W1T)
    shared = {
        "w2h01": np.ascontiguousarray(W2Th[0:256]),
        "w2h2": np.ascontiguousarray(W2Th[256:300]),
        "w2l01": np.ascontiguousarray(W2Tl[0:256]),
        "w2l2": np.ascontiguousarray(W2Tl[256:300]),
        "w1h": W1Th,
        "w1l": W1Tl,
        "bias_row": np.asarray(attn_b, dtype=np.float32).reshape(1, H),
        "vbc": np.broadcast_to(
            np.asarray(v, dtype=np.float32).reshape(1, H), (128, H)
        ).copy(),
        "ident": np.eye(128, dtype=np.float32),
        "ones128": np.ones((128, 128), dtype=np.float32),
    }

    in_maps = []
    enc_all = np.asarray(encoder_outputs, dtype=np.float32)
    lh_all = np.asarray(last_hidden, dtype=np.float32)
    for c in range(NCORES):
        ids = assign[c]
        enc_c = np.ascontiguousarray(
            np.transpose(enc_all[:, ids, :], (1, 0, 2)))      # [BL, L, H] f32
        eh, el = _split_bf16(enc_c)
        ehT = np.empty((BL, H + 1, L), dtype=bf)
        ehT[:, 0:H, :] = np.transpose(eh, (0, 2, 1))
        ehT[:, H, :] = np.float32(1.0)
        elT = np.ascontiguousarray(np.transpose(el, (0, 2, 1)))
        lhT = np.ascontiguousarray(lh_all[ids].T)             # [H, BL] f32
        lhh, lhl = _split_bf16(lhT)
        pen_np = np.zeros((128, n_mask), dtype=np.float32)
        for col, (s, t) in enumerate(pen_cols):
            b = ids[s]
            lo = int(src_len[b]) - t * TP
            lo = max(0, min(128, lo))
            pen_np[lo:, col] = NEG
        m = dict(shared)
        m["enc16"] = eh
        m["ehT"] = ehT
        m["elT"] = elT
        m["lhh"] = lhh
        m["lhl"] = lhl
        m["pen"] = pen_np
        in_maps.append(m)

    return C, mask_col, n_mask, in_maps, assign


_CACHE = {}
TRACE = False           # set by test harness to capture neuron-profile timing
LAST_EXEC_NS = None
LAST_RESULTS = None


def _ensure_ntff_hook():
    """The agent image's antenv lacks axon_hooks; synthesize it so
    run_bass_kernel_spmd(trace=True) can capture NTFF profiles."""
    import types
    try:
        from antenv.axon_hooks import get_axon_ntff_profile_hook  # noqa: F401
        return
    except ImportError:
        pass
    try:
        from trn_agent_boot.trn_boot import _ntff_profile_via_ctypes
    except ImportError:
        return
    so = "/opt/axon/libaxon_pjrt.so"
    if not os.path.exists(so):
        return
    hook = _ntff_profile_via_ctypes(so)
    mod = types.ModuleType("antenv.axon_hooks")
    mod._hook = hook
    mod.get_axon_ntff_profile_hook = lambda: mod._hook
    mod.set_axon_ntff_profile_hook = lambda h: setattr(mod, "_hook", h)
    import antenv
    sys.modules["antenv.axon_hooks"] = mod
    antenv.axon_hooks = mod


def kernel(last_hidden, encoder_outputs, attn_W, attn_b, v,
           Wp_W=None, Wp_b=None, v_p=None, src_len=None, **_unused):
    C, mask_col, n_mask, in_maps, assign = _prep(
        last_hidden, encoder_outputs, attn_W, attn_b, v, src_len
    )
    nb_enc = min(40, max(C) + 20)

    key = (tuple(C), tuple(sorted(mask_col.items())), n_mask, nb_enc)
    nc = _CACHE.get(key)
    if nc is None:
        nc = _build_program(C, mask_col, n_mask, nb_enc)
        _CACHE[key] = nc

    global LAST_EXEC_NS, LAST_RESULTS
    if TRACE:
        _ensure_ntff_hook()
    res = run_bass_kernel_spmd(nc, in_maps, core_ids=list(range(NCORES)),
                               trace=TRACE)
    LAST_EXEC_NS = res.exec_time_ns
    LAST_RESULTS = res

    ctx_full = np.zeros((B, 1, H), dtype=np.float32)
    attn_full = np.zeros((B, 1, L), dtype=np.float32)
    for c in range(NCORES):
        out = res.results[c]
        for s in range(BL):
            b = assign[c, s]
            ctx_full[b, 0, :] = out["ctx_out"][s]
            attn_full[b, 0, :] = out["attn_out"][s]
    return ctx_full, attn_full


# revision 16
# speedup vs baseline: 3.1799x; 3.1799x over previous
"""Trainium2 Bass kernel for additive (Bahdanau-style) attention with length masking.

Reference computation (per batch b):
    q[b]      = W1 @ last_hidden[b] + attn_b                  # [H]
    k[l,b]    = W2 @ encoder_outputs[l,b]                     # [H]
    score[b,l]= v . tanh(q[b] + k[l,b])                       # masked to l < src_len[b]
    attn[b]   = softmax_l(score[b])                           # [L]
    ctx[b]    = sum_l attn[b,l] * encoder_outputs[l,b]        # [H]
returns (ctx [B,1,H], attn [B,1,L])

Sharding: data-parallel over batch. 128 batches -> 8 cores x 16 slots.
Batches are sorted by ceil(src_len/128) and dealt rank-major so all cores
run one identical program with per-slot tile counts C[s]; rows beyond
src_len are masked, tiles beyond ceil(src_len/128) are skipped entirely.

Precision: the score matmul runs as a two-term compensated bf16 product
    e @ W2^T ~= eh@Wh + eh@Wl + el@Wh      (eh,el / Wh,Wl bf16 hi/lo splits)
because this hardware path supports only bf16 (1 cyc/row) and fp32
(4 cyc/row) matmuls -- fp16 / f32r / int16 all fail in walrus or on
silicon. Everything after the matmul stays fp32. The q projection uses
the same compensated split (one-time). The q+bias row rides inside the
third K-chunk of the score matmul via a ones-row appended to the
transposed operand (host-prepared).
"""

import sys
import os

if "/opt/trn_rl_repo" not in sys.path:
    sys.path.insert(0, "/opt/trn_rl_repo")

import numpy as np
import ml_dtypes

from concourse import bass, bacc, tile, mybir
from concourse.bass_utils import run_bass_kernel_spmd

F32 = mybir.dt.float32
BF16 = mybir.dt.bfloat16
AF = mybir.ActivationFunctionType
ALU = mybir.AluOpType

H = 300
L = 2048
B = 128
NCORES = 8
BL = B // NCORES        # batches (slots) per core
TP = 128                # rows per tile
NEG = -1.0e9


def _build_program(C, mask_col, n_mask, nb_enc):
    nc = bacc.Bacc("TRN2", target_bir_lowering=False, debug=False)

    enc_d = nc.declare_dram_parameter("enc16", [BL, L, H], BF16, isOutput=False)
    ehlT_d = nc.declare_dram_parameter("ehlT", [BL, 128, 5, L], BF16, isOutput=False)
    w2h01_d = nc.declare_dram_parameter("w2h01", [256, H], BF16, isOutput=False)
    w2h2_d = nc.declare_dram_parameter("w2h2", [44, H], BF16, isOutput=False)
    w2l01_d = nc.declare_dram_parameter("w2l01", [256, H], BF16, isOutput=False)
    w2l2_d = nc.declare_dram_parameter("w2l2", [44, H], BF16, isOutput=False)
    w1h_d = nc.declare_dram_parameter("w1h", [H, H], BF16, isOutput=False)
    w1l_d = nc.declare_dram_parameter("w1l", [H, H], BF16, isOutput=False)
    lhh_d = nc.declare_dram_parameter("lhh", [H, BL], BF16, isOutput=False)
    lhl_d = nc.declare_dram_parameter("lhl", [H, BL], BF16, isOutput=False)
    bias_d = nc.declare_dram_parameter("bias_row", [1, H], F32, isOutput=False)
    vbc_d = nc.declare_dram_parameter("vbc", [128, H], F32, isOutput=False)
    ident_d = nc.declare_dram_parameter("ident", [128, 128], F32, isOutput=False)
    ones_d = nc.declare_dram_parameter("ones128", [128, 128], F32, isOutput=False)
    pen_d = nc.declare_dram_parameter("pen", [128, n_mask], F32, isOutput=False)

    ctx_d = nc.declare_dram_parameter("ctx_out", [BL, H], F32, isOutput=True)
    attn_d = nc.declare_dram_parameter("attn_out", [BL, L], F32, isOutput=True)

    with tile.TileContext(nc) as tc:
        with (
            tc.tile_pool(name="const", bufs=1) as constp,
            tc.tile_pool(name="s3", bufs=1) as s3p,
            tc.tile_pool(name="encring", bufs=1) as encp,
            tc.tile_pool(name="encT", bufs=12) as encTp,
            tc.tile_pool(name="th", bufs=3) as thp,
            tc.tile_pool(name="ttr", bufs=2) as ttrp,
            tc.tile_pool(name="scores", bufs=2) as scorep,
            tc.tile_pool(name="wts", bufs=2) as wp,
            tc.tile_pool(name="small", bufs=4) as smallp,
            tc.tile_pool(name="mm1ps", bufs=2, space="PSUM") as mm1ps,
            tc.tile_pool(name="ctxps", bufs=2, space="PSUM") as ctxps,
            tc.tile_pool(name="sps", bufs=3, space="PSUM") as sps,
        ):
            # ---- constants ----
            w2h0 = constp.tile([128, H], BF16, tag="w2h0")
            w2h1 = constp.tile([128, H], BF16, tag="w2h1")
            w2h2 = constp.tile([44, H], BF16, tag="w2h2")
            w2l0 = constp.tile([128, H], BF16, tag="w2l0")
            w2l1 = constp.tile([128, H], BF16, tag="w2l1")
            vbc = constp.tile([128, H], F32, tag="vbc")
            ident = constp.tile([128, 128], F32, tag="ident")
            ones = constp.tile([128, 128], F32, tag="ones")
            pen = constp.tile([128, n_mask], F32, tag="pen")
            w1h = [constp.tile([min(128, H - 128 * i), H], BF16,
                               tag=f"w1h{i}", name=f"w1h{i}") for i in range(3)]
            w1l = [constp.tile([min(128, H - 128 * i), H], BF16,
                               tag=f"w1l{i}", name=f"w1l{i}") for i in range(3)]
            lhh = [constp.tile([min(128, H - 128 * i), BL], BF16,
                               tag=f"lhh{i}", name=f"lhh{i}") for i in range(3)]
            lhl = [constp.tile([min(128, H - 128 * i), BL], BF16,
                               tag=f"lhl{i}", name=f"lhl{i}") for i in range(3)]
            biasr = constp.tile([1, H], F32, tag="biasr")
            qh16 = constp.tile([BL, H], BF16, tag="qh16")
            ql16 = constp.tile([BL, H], BF16, tag="ql16")

            nc.sync.dma_start(w2h0[:], w2h01_d[0:128, :])
            nc.sync.dma_start(w2h1[:], w2h01_d[128:256, :])
            nc.sync.dma_start(w2l0[:], w2l01_d[0:128, :])
            nc.sync.dma_start(w2l1[:], w2l01_d[128:256, :])
            for i, (k0, k1) in enumerate([(0, 128), (128, 256), (256, 300)]):
                nc.sync.dma_start(w1h[i][:], w1h_d[k0:k1, :])
                nc.sync.dma_start(w1l[i][:], w1l_d[k0:k1, :])
                nc.sync.dma_start(lhh[i][:], lhh_d[k0:k1, :])
                nc.sync.dma_start(lhl[i][:], lhl_d[k0:k1, :])
            for tl, dd in [(w2h2, w2h2_d), (vbc, vbc_d), (ident, ident_d),
                           (ones, ones_d), (pen, pen_d), (biasr, bias_d)]:
                nc.sync.dma_start(tl[:], dd[:])

            # slot-alternating rhs blocks for the merged tail K-chunk.
            # s3h [89, H]: rows 0:44 = W2h tail, row 44 = qh[slot],
            #              rows 45:89 = W2h tail again (for the el part)
            # s3l [45, H]: rows 0:44 = W2l tail, row 44 = ql[slot]
            s3h = [s3p.tile([89, H], BF16, tag=f"s3h_{i}", name=f"s3h_{i}")
                   for i in range(2)]
            s3l = [s3p.tile([45, H], BF16, tag=f"s3l_{i}", name=f"s3l_{i}")
                   for i in range(2)]
            for i in range(2):
                nc.sync.dma_start(s3h[i][0:44, :], w2h2_d[:])
                nc.sync.dma_start(s3h[i][45:89, :], w2h2_d[:])
                nc.sync.dma_start(s3l[i][0:44, :], w2l2_d[:])

            # ---- q = last_hidden @ W1^T + attn_b, compensated bf16 ----
            q_ps = sps.tile([BL, H], F32, tag="sp")
            first = True
            for (lh_t, w_t) in [(lhh, w1h), (lhh, w1l), (lhl, w1h)]:
                for i in range(3):
                    nc.tensor.matmul(q_ps[:], lh_t[i][:], w_t[i][:],
                                     start=first, stop=False)
                    first = False
            nc.tensor.matmul(q_ps[:], ones[0:1, 0:BL], biasr[:],
                             start=False, stop=True)
            q32 = constp.tile([BL, H], F32, tag="q32")
            nc.vector.tensor_copy(q32[:], q_ps[:])
            nc.vector.tensor_copy(qh16[:], q32[:])
            qh32 = constp.tile([BL, H], F32, tag="qh32")
            nc.vector.tensor_copy(qh32[:], qh16[:])
            ql32 = constp.tile([BL, H], F32, tag="ql32")
            nc.vector.tensor_tensor(out=ql32[:], in0=q32[:], in1=qh32[:],
                                    op=ALU.subtract)
            nc.vector.tensor_copy(ql16[:], ql32[:])

            # ---- native-layout enc ring (resident for the ctx matmul) ----
            ebs16 = [encp.tile([TP, H], BF16, tag=f"eb16_{i}", name=f"eb16_{i}")
                     for i in range(nb_enc)]

            gidx = 0
            tile_of = {}
            for s in range(BL):
                cs = C[s]
                s3hb = s3h[s % 2]
                s3lb = s3l[s % 2]
                nc.sync.dma_start(s3hb[44:45, :], qh16[s:s + 1, :])
                nc.sync.dma_start(s3lb[44:45, :], ql16[s:s + 1, :])

                scores = scorep.tile([128, BL], F32, tag="scores")
                for t in range(cs):
                    eb16 = ebs16[gidx % nb_enc]
                    tile_of[(s, t)] = eb16
                    gidx += 1
                    nc.scalar.dma_start(eb16[:], enc_d[s][t * TP:(t + 1) * TP, :])
                    eT = encTp.tile([128, 640], BF16, tag="ehl", name="ehl")
                    nc.sync.dma_start(
                        eT[:].rearrange("p (k l) -> p k l", k=5),
                        ehlT_d[s][:, :, t * TP:(t + 1) * TP])
                    pe = mm1ps.tile([TP, H], F32, tag="mm1")
                    nc.tensor.matmul(pe[:], eT[:, 0:128], w2h0[:],
                                     start=True, stop=False)
                    nc.tensor.matmul(pe[:], eT[:, 0:128], w2l0[:],
                                     start=False, stop=False)
                    nc.tensor.matmul(pe[:], eT[:, 128:256], w2h1[:],
                                     start=False, stop=False)
                    nc.tensor.matmul(pe[:], eT[:, 128:256], w2l1[:],
                                     start=False, stop=False)
                    nc.tensor.matmul(pe[:], eT[0:45, 256:384], s3lb[0:45, :],
                                     start=False, stop=False)
                    nc.tensor.matmul(pe[:], eT[0:89, 256:384], s3hb[0:89, :],
                                     start=False, stop=False)
                    nc.tensor.matmul(pe[:], eT[:, 384:512], w2h0[:],
                                     start=False, stop=False)
                    nc.tensor.matmul(pe[:], eT[:, 512:640], w2h1[:],
                                     start=False, stop=True)
                    th = thp.tile([TP, H], F32, tag="th")
                    nc.scalar.activation(th[:], pe[:], AF.Tanh)
                    td = ttrp.tile([TP, H], F32, tag="ttr")
                    nc.vector.tensor_tensor(out=td[:], in0=th[:], in1=vbc[:],
                                            op=ALU.mult)
                    nc.vector.tensor_reduce(scores[:, t:t + 1], td[:],
                                            axis=mybir.AxisListType.X,
                                            op=ALU.add)
                    pcol = mask_col.get((s, t))
                    if pcol is not None:
                        nc.vector.tensor_tensor(
                            out=scores[:, t:t + 1], in0=scores[:, t:t + 1],
                            in1=pen[:, pcol:pcol + 1], op=ALU.add)

                # ---- softmax over this slot's cs*128 scores ----
                m1 = smallp.tile([128, 1], F32, tag="m1")
                nc.vector.tensor_reduce(m1[:], scores[:, 0:cs],
                                        axis=mybir.AxisListType.X, op=ALU.max)
                mT = sps.tile([1, 128], F32, tag="sp")
                nc.tensor.matmul(mT[:], m1[:], ident[:], is_transpose=True,
                                 start=True, stop=True)
                mmax = smallp.tile([1, 1], F32, tag="mmax")
                nc.vector.tensor_reduce(mmax[:], mT[:],
                                        axis=mybir.AxisListType.X, op=ALU.max)
                negM = smallp.tile([1, 1], F32, tag="negM")
                nc.vector.tensor_scalar_mul(negM[:], mmax[:], -1.0)
                nMb_ps = sps.tile([128, 1], F32, tag="sp")
                nc.tensor.matmul(nMb_ps[:], ones[0:1, :], negM[:],
                                 start=True, stop=True)
                negMb = smallp.tile([128, 1], F32, tag="negMb")
                nc.vector.tensor_copy(negMb[:], nMb_ps[:])

                wts = wp.tile([128, BL], F32, tag="wts")
                sume = smallp.tile([128, 1], F32, tag="sume")
                nc.scalar.activation(wts[:, 0:cs], scores[:, 0:cs], AF.Exp,
                                     bias=negMb[:], scale=1.0)
                nc.vector.tensor_reduce(sume[:], wts[:, 0:cs],
                                        axis=mybir.AxisListType.X, op=ALU.add)
                z_ps = sps.tile([1, 1], F32, tag="sp")
                nc.tensor.matmul(z_ps[:], ones[:, 0:1], sume[:],
                                 start=True, stop=True)
                zsb = smallp.tile([1, 1], F32, tag="zsb")
                nc.vector.tensor_copy(zsb[:], z_ps[:])
                rinv = smallp.tile([1, 1], F32, tag="rinv")
                nc.vector.reciprocal(rinv[:], zsb[:])
                rb_ps = sps.tile([128, 1], F32, tag="sp")
                nc.tensor.matmul(rb_ps[:], ones[0:1, :], rinv[:],
                                 start=True, stop=True)
                rinvb = smallp.tile([128, 1], F32, tag="rinvb")
                nc.vector.tensor_copy(rinvb[:], rb_ps[:])

                # ---- ctx[s] = (1/Z) * sum_t enc_tile^T @ w_tile (bf16 PE) ----
                w16 = wp.tile([128, BL], BF16, tag="w16")
                nc.vector.tensor_copy(w16[:, 0:cs], wts[:, 0:cs])
                cps = ctxps.tile([1, H], F32, tag="ctx")
                for t in range(cs):
                    eb16 = tile_of[(s, t)]
                    nc.tensor.matmul(
                        cps[:],
                        w16[:, t:t + 1],
                        eb16[:],
                        start=(t == 0), stop=(t == cs - 1),
                    )
                ctx_sb = smallp.tile([1, H], F32, tag="ctxsb")
                nc.scalar.activation(ctx_sb[:], cps[:], AF.Copy, scale=rinv[:])
                nc.sync.dma_start(ctx_d[s:s + 1, :], ctx_sb[:])

                # ---- attn weights out: normalize, transpose, store ----
                wn = wp.tile([128, BL], F32, tag="wn")
                nc.scalar.activation(wn[:, 0:cs], wts[:, 0:cs], AF.Copy,
                                     scale=rinvb[:])
                aT_ps = sps.tile([BL, 128], F32, tag="sp")
                nc.tensor.matmul(aT_ps[0:cs, :], wn[:, 0:cs], ident[:],
                                 is_transpose=True, start=True, stop=True)
                aT = smallp.tile([BL, 128], F32, tag="aT")
                nc.vector.tensor_copy(aT[0:cs, :], aT_ps[0:cs, :])
                nc.sync.dma_start(
                    attn_d[s].rearrange("(t l) -> t l", l=TP)[0:cs, :],
                    aT[0:cs, :],
                )

    nc.compile()
    return nc


def _split_bf16(x):
    bf = ml_dtypes.bfloat16
    hi = x.astype(bf)
    lo = (x - hi.astype(np.float32)).astype(bf)
    return hi, lo


def _prep(last_hidden, encoder_outputs, attn_W, attn_b, v, src_len):
    """Host-side sharding / marshaling."""
    bf = ml_dtypes.bfloat16
    src_len = np.asarray(src_len)
    tiles = -(-src_len // TP)                      # ceil
    order = np.argsort(-tiles, kind="stable")
    assign = np.zeros((NCORES, BL), dtype=np.int64)
    C = []
    for s in range(BL):
        grp = order[NCORES * s:NCORES * (s + 1)]
        assign[:, s] = grp
        C.append(int(tiles[grp].max()))

    mask_col = {}
    pen_cols = []
    for s in range(BL):
        for t in range(C[s]):
            needs = False
            for c in range(NCORES):
                b = assign[c, s]
                if (t + 1) * TP > src_len[b]:
                    needs = True
                    break
            if needs:
                mask_col[(s, t)] = len(pen_cols)
                pen_cols.append((s, t))
    n_mask = max(1, len(pen_cols))

    W1 = np.asarray(attn_W[:, :H], dtype=np.float32)   # [h_out, h_in]
    W2 = np.asarray(attn_W[:, H:], dtype=np.float32)
    W2T = np.ascontiguousarray(W2.T)                   # [h_in, h_out]
    W1T = np.ascontiguousarray(W1.T)
    W2Th, W2Tl = _split_bf16(W2T)
    W1Th, W1Tl = _split_bf16(W1T)
    shared = {
        "w2h01": np.ascontiguousarray(W2Th[0:256]),
        "w2h2": np.ascontiguousarray(W2Th[256:300]),
        "w2l01": np.ascontiguousarray(W2Tl[0:256]),
        "w2l2": np.ascontiguousarray(W2Tl[256:300]),
        "w1h": W1Th,
        "w1l": W1Tl,
        "bias_row": np.asarray(attn_b, dtype=np.float32).reshape(1, H),
        "vbc": np.broadcast_to(
            np.asarray(v, dtype=np.float32).reshape(1, H), (128, H)
        ).copy(),
        "ident": np.eye(128, dtype=np.float32),
        "ones128": np.ones((128, 128), dtype=np.float32),
    }

    in_maps = []
    enc_all = np.asarray(encoder_outputs, dtype=np.float32)
    lh_all = np.asarray(last_hidden, dtype=np.float32)
    for c in range(NCORES):
        ids = assign[c]
        enc_c = np.ascontiguousarray(
            np.transpose(enc_all[:, ids, :], (1, 0, 2)))      # [BL, L, H] f32
        eh, el = _split_bf16(enc_c)
        ehT = np.transpose(eh, (0, 2, 1))          # [BL, H, L]
        elT = np.transpose(el, (0, 2, 1))
        ehlT = np.zeros((BL, 128, 5, L), dtype=bf)
        ehlT[:, :, 0, :] = ehT[:, 0:128, :]
        ehlT[:, :, 1, :] = ehT[:, 128:256, :]
        ehlT[:, 0:44, 2, :] = ehT[:, 256:300, :]
        ehlT[:, 44, 2, :] = np.float32(1.0)        # ones row (q rides here)
        ehlT[:, 45:89, 2, :] = elT[:, 256:300, :]
        ehlT[:, :, 3, :] = elT[:, 0:128, :]
        ehlT[:, :, 4, :] = elT[:, 128:256, :]
        lhT = np.ascontiguousarray(lh_all[ids].T)             # [H, BL] f32
        lhh, lhl = _split_bf16(lhT)
        pen_np = np.zeros((128, n_mask), dtype=np.float32)
        for col, (s, t) in enumerate(pen_cols):
            b = ids[s]
            lo = int(src_len[b]) - t * TP
            lo = max(0, min(128, lo))
            pen_np[lo:, col] = NEG
        m = dict(shared)
        m["enc16"] = eh
        m["ehlT"] = ehlT
        m["lhh"] = lhh
        m["lhl"] = lhl
        m["pen"] = pen_np
        in_maps.append(m)

    return C, mask_col, n_mask, in_maps, assign


_CACHE = {}
TRACE = False           # set by test harness to capture neuron-profile timing
LAST_EXEC_NS = None
LAST_RESULTS = None


def _ensure_ntff_hook():
    """The agent image's antenv lacks axon_hooks; synthesize it so
    run_bass_kernel_spmd(trace=True) can capture NTFF profiles."""
    import types
    try:
        from antenv.axon_hooks import get_axon_ntff_profile_hook  # noqa: F401
        return
    except ImportError:
        pass
    try:
        from trn_agent_boot.trn_boot import _ntff_profile_via_ctypes
    except ImportError:
        return
    so = "/opt/axon/libaxon_pjrt.so"
    if not os.path.exists(so):
        return
    hook = _ntff_profile_via_ctypes(so)
    mod = types.ModuleType("antenv.axon_hooks")
    mod._hook = hook
    mod.get_axon_ntff_profile_hook = lambda: mod._hook
    mod.set_axon_ntff_profile_hook = lambda h: setattr(mod, "_hook", h)
    import antenv
    sys.modules["antenv.axon_hooks"] = mod
    antenv.axon_hooks = mod


def kernel(last_hidden, encoder_outputs, attn_W, attn_b, v,
           Wp_W=None, Wp_b=None, v_p=None, src_len=None, **_unused):
    C, mask_col, n_mask, in_maps, assign = _prep(
        last_hidden, encoder_outputs, attn_W, attn_b, v, src_len
    )
    nb_enc = min(40, max(C) + 20)

    key = (tuple(C), tuple(sorted(mask_col.items())), n_mask, nb_enc)
    nc = _CACHE.get(key)
    if nc is None:
        nc = _build_program(C, mask_col, n_mask, nb_enc)
        _CACHE[key] = nc

    global LAST_EXEC_NS, LAST_RESULTS
    if TRACE:
        _ensure_ntff_hook()
    res = run_bass_kernel_spmd(nc, in_maps, core_ids=list(range(NCORES)),
                               trace=TRACE)
    LAST_EXEC_NS = res.exec_time_ns
    LAST_RESULTS = res

    ctx_full = np.zeros((B, 1, H), dtype=np.float32)
    attn_full = np.zeros((B, 1, L), dtype=np.float32)
    for c in range(NCORES):
        out = res.results[c]
        for s in range(BL):
            b = assign[c, s]
            ctx_full[b, 0, :] = out["ctx_out"][s]
            attn_full[b, 0, :] = out["attn_out"][s]
    return ctx_full, attn_full
